# revision 1
# baseline (speedup 1.0000x reference)
"""Causal self-attention (RoPE) Trainium2 kernel, 8-way token-parallel.

Contract: kernel(**inputs) takes the full unsharded inputs
  x [B,T,C] f32, W_attn [C,3C] f32, W_proj [C,C] f32,
  rope_cos/rope_sin [T,D/2] f32, attention_mask [B,1,T,T] bool (all ones)
and returns the full output [B,T,C] f32.

Sharding: 8 cores; core m handles Tq = B*T/8 consecutive tokens of batch
m//(cores_per_batch). Each core redundantly computes k/v for its whole
batch (no collectives needed), attention for its query slice over all
keys (causality via a per-core 0/1 mask multiplied into exp(S)), then the
output projection for its slice. The host concatenates the slices.

On-chip layout is fully transposed (channels on partitions, tokens on the
free dim): qT/kT = [D, tokens] per head, v = [keys, D] per head,
S^T = [keys, q] so the PV matmul needs no transposes and the softmax
denominator is a ones-matmul (which also broadcasts it across partitions).
RoPE: W_attn q/k columns are permuted per head to (64 reals | 64 imags);
the rotation is x*cc + swap64(x)*ss with host-precomputed cc/ss tables,
where swap64 is two cross-partition ScalarE copies out of PSUM.

All matmul operands are float32r (~1e-4 matmul rel-err at bf16-class
speed). Weights / x are pre-arranged on the host so every SBUF tile load
is one contiguous DMA (descriptor count, not bytes, is the DMA-queue
bottleneck). Loads go on the sync queue, stores on gpsimd.
"""

import sys

sys.path.insert(0, "/opt/trn_rl_repo")

import numpy as np

import concourse.bacc as bacc
import concourse.bass as bass
import concourse.mybir as mybir
import concourse.tile as tile
from concourse.bass_utils import run_bass_kernel_spmd

F32 = mybir.dt.float32
F32R = mybir.dt.float32r
BF16 = mybir.dt.bfloat16
MM_DT = BF16          # matmul operand dtype (2-byte: x/W stay SBUF-resident)
N_CORES = 8


def build_nc(B, T, C, Tq, repeat=1, phases="ABC"):
    """One core's program. H = C//128 heads, D = 128.

    Per-core inputs (host pre-arranged, see _prep_inputs):
      xT    [C, T]            x[b].T (k/q projection moving operand)
      xTq   [C, Tq]           slice of xT for this core's query tokens
      x_v   [nkc, 128, ncc, 128]   x re-tiled for the v projection
      Wq,Wk [H, 128, ncc, 128]     rope-permuted, tiled per head
      Wv    [nvg, 128, ncc, 512]   tiled per 512-col group
      Wp    [ncc, 128, H, 128]     tiled per 128-col out group
      cc,ss [128, T] f32      rope cos / (-sin|+sin) tables
      ccq,ssq [128, Tq] f32   same, sliced to the query tokens
      mask  [128, nkc, Tq] f32   0/1 causal mask, keys on partitions
      ones  [128, 128]        all-ones (softmax denominator matmul)
    Output:
      outT  [C, Tq] f32
    """
    D = 128
    H = C // D
    nkc = T // 128        # key chunks
    ntt = T // 512        # 512-wide token tiles
    ncc = C // 128        # contraction chunks
    nvg = C // 256
    assert Tq % 128 == 0 and T % 512 == 0 and C % 512 == 0

    nc = bacc.Bacc(None)
    xT = nc.dram_tensor("xT", [C, T], MM_DT, kind="ExternalInput")
    Wq = nc.dram_tensor("Wq", [H, 128, ncc, 128], MM_DT, kind="ExternalInput")
    Wk = nc.dram_tensor("Wk", [H, 128, ncc, 128], MM_DT, kind="ExternalInput")
    Wv = nc.dram_tensor("Wv", [nvg, 128, ncc, 256], MM_DT, kind="ExternalInput")
    Wp = nc.dram_tensor("Wp", [ncc, 128, H, 128], MM_DT, kind="ExternalInput")
    cc = nc.dram_tensor("cc", [128, T], BF16, kind="ExternalInput")
    ss = nc.dram_tensor("ss", [128, T], BF16, kind="ExternalInput")
    mask = nc.dram_tensor("mask", [128, nkc, Tq], MM_DT, kind="ExternalInput")
    ones = nc.dram_tensor("ones", [128, 128], MM_DT, kind="ExternalInput")
    outT = nc.dram_tensor("outT", [C, Tq], F32, kind="ExternalOutput")

    xT_ch = xT.rearrange("(cc p) x -> p cc x", p=128)

    kT_dram = nc.dram_tensor("kT_stage", [ntt, H, 128, 512], MM_DT)
    v_dram = nc.dram_tensor("v_stage", [nkc, 128, C], MM_DT)

    scale = 1.0 / float(np.sqrt(np.float32(D)))

    with tile.TileContext(nc) as tc:
        from contextlib import ExitStack
        es_pools = ExitStack()
        with es_pools:
          # -------- persistent pools (whole kernel) --------
          p_qT = es_pools.enter_context(tc.tile_pool(name="qT", bufs=1))
          p_y = es_pools.enter_context(tc.tile_pool(name="y", bufs=1))
          p_ps = es_pools.enter_context(tc.tile_pool(name="ps", bufs=4, space="PSUM"))
          p_acc = es_pools.enter_context(tc.tile_pool(name="ps_acc", bufs=2, space="PSUM"))
          for _rep in range(repeat):
              qT_sb = p_qT.tile([128, H, Tq], MM_DT, tag="qT")
              y_sb = p_y.tile([128, H, Tq], MM_DT, tag="y")

              # ================ phase A: qkv projection + rope ================
              assert mybir.dt.size(MM_DT) == 2, "phase A needs x/W resident (2-byte dtype)"
              with (
                  tc.tile_pool(name="tabs", bufs=1) as p_tabs,
                  tc.tile_pool(name="xall", bufs=1) as p_xall,
                  tc.tile_pool(name="wres", bufs=1) as p_wres,
                  tc.tile_pool(name="wv2", bufs=2) as p_wv2,
                  tc.tile_pool(name="wstr", bufs=2) as p_wstr,
                  tc.tile_pool(name="rope", bufs=2) as p_rope,
                  tc.tile_pool(name="stage", bufs=3) as p_stage,
              ):
                  cc_sb = p_tabs.tile([128, T], BF16, tag="cc")
                  ss_sb = p_tabs.tile([128, T], BF16, tag="ss")
                  nc.sync.dma_start(cc_sb[:], cc[:])
                  nc.sync.dma_start(ss_sb[:], ss[:])

                  def rope(dst_ap, src_ps, cc_t, ss_t, n):
                      """dst = src*cc + swap64(src)*ss; src stays in PSUM."""
                      sw = p_rope.tile([128, n], F32, tag="rp_sw")
                      nc.scalar.copy(sw[0:64, :], src_ps[64:128, :])
                      nc.scalar.copy(sw[64:128, :], src_ps[0:64, :])
                      t1 = p_rope.tile([128, n], F32, tag="rp_t1")
                      nc.vector.tensor_mul(t1[:], src_ps[:], cc_t)
                      nc.vector.tensor_mul(sw[:], sw[:], ss_t)
                      nc.vector.tensor_add(dst_ap, t1[:], sw[:])

                  # x and Wk resident for the whole phase
                  xall_t = []
                  for tt in range(ntt):
                      xt = p_xall.tile([128, ncc, 512], MM_DT, tag=f"xall{tt}")
                      eng = nc.sync if tt % 2 == 0 else nc.scalar
                      eng.dma_start(xt[:], xT_ch[:, :, tt * 512:(tt + 1) * 512])
                      xall_t.append(xt)

                  def xall_col(c, j0, n):
                      tt, o = divmod(j0, 512)
                      assert o + n <= 512
                      return xall_t[tt][:, c, o:o + n]
                  wk_all = p_wres.tile([128, H, ncc, 128], MM_DT, tag="wk_all")
                  nc.sync.dma_start(wk_all[:], Wk.rearrange("h p cc d -> p h cc d"))

                  # ---- q projection + rope ----
                  for h in range(H):
                      wq_h = p_wstr.tile([128, ncc, 128], MM_DT, tag="wq")
                      nc.scalar.dma_start(wq_h[:], Wq[h])
                      q_ps = p_ps.tile([128, Tq], F32, tag="mm")
                      for c in range(ncc):
                          nc.tensor.matmul(
                              q_ps[:], wq_h[:, c, :], xall_col(c, 0, Tq),
                              start=(c == 0), stop=(c == ncc - 1),
                          )
                      rope(qT_sb[:, h, :], q_ps, cc_sb[:, 0:Tq], ss_sb[:, 0:Tq], Tq)

                  # ---- k projection + rope -> DRAM ----
                  for h in range(H):
                      for tt in range(ntt):
                          k_ps = p_ps.tile([128, 512], F32, tag="mm")
                          for c in range(ncc):
                              nc.tensor.matmul(
                                  k_ps[:], wk_all[:, h, c, :],
                                  xall_col(c, tt * 512, 512),
                                  start=(c == 0), stop=(c == ncc - 1),
                              )
                          kst = p_stage.tile([128, 512], MM_DT, tag="kst")
                          rope(kst[:], k_ps, cc_sb[:, tt * 512:(tt + 1) * 512],
                               ss_sb[:, tt * 512:(tt + 1) * 512], 512)
                          nc.gpsimd.dma_start(kT_dram[tt, h], kst[:])

                  # ---- v projection -> DRAM ----
                  for vg in range(nvg):
                      wv_g = p_wv2.tile([128, ncc, 256], MM_DT, tag="wv")
                      nc.sync.dma_start(wv_g[:], Wv[vg])
                      for kc in range(nkc):
                          v_ps = p_ps.tile([128, 256], F32, tag="mm")
                          for c in range(ncc):
                              nc.tensor.matmul(
                                  v_ps[:], xall_col(c, kc * 128, 128),
                                  wv_g[:, c, :],
                                  start=(c == 0), stop=(c == ncc - 1),
                              )
                          vst = p_stage.tile([128, 256], MM_DT, tag="vst")
                          nc.vector.tensor_copy(vst[:], v_ps[:])
                          nc.gpsimd.dma_start(
                              v_dram[kc, :, vg * 256:(vg + 1) * 256], vst[:]
                          )

              # ================ phase B: attention ================
              if "B" not in phases:
                  continue
              with (
                  tc.tile_pool(name="battn", bufs=2) as p_attn,
                  tc.tile_pool(name="bes", bufs=3) as p_es,
                  tc.tile_pool(name="bmask", bufs=1) as p_mask,
              ):
                  mask_sb = p_mask.tile([128, nkc, Tq], MM_DT, tag="mask")
                  nc.sync.dma_start(mask_sb[:], mask[:])
                  ones_sb = p_mask.tile([128, 128], MM_DT, tag="ones")
                  nc.sync.dma_start(ones_sb[:], ones[:])
                  for h in range(H):
                      kT_h = p_attn.tile([128, T], MM_DT, tag="kT_h")
                      nc.sync.dma_start(kT_h.rearrange("p (tt x) -> p tt x", x=512), kT_dram[:, h].rearrange("tt p x -> p tt x"))
                      v_h = p_attn.tile([128, nkc, 128], MM_DT, tag="v_h")
                      nc.sync.dma_start(v_h[:], v_dram[:, :, h * 128:(h + 1) * 128].rearrange("kc p d -> p kc d"))
                      y_ps = p_acc.tile([128, Tq], F32, tag="y_acc")
                      cs_ps = p_acc.tile([128, Tq], F32, tag="cs_acc")
                      for kc in range(nkc):
                          s_ps = p_ps.tile([128, Tq], F32, tag="mm")
                          nc.tensor.matmul(
                              s_ps[:], kT_h[:, kc * 128:(kc + 1) * 128], qT_sb[:, h, :],
                              start=True, stop=True,
                          )
                          es = p_es.tile([128, Tq], MM_DT, tag="es")
                          nc.scalar.activation(
                              es[:], s_ps[:], mybir.ActivationFunctionType.Exp, scale=scale
                          )
                          em = p_es.tile([128, Tq], MM_DT, tag="em")
                          nc.vector.tensor_mul(em[:], es[:], mask_sb[:, kc, :])
                          nc.tensor.matmul(
                              cs_ps[:], ones_sb[:], em[:],
                              start=(kc == 0), stop=(kc == nkc - 1),
                          )
                          nc.tensor.matmul(
                              y_ps[:], v_h[:, kc, :], em[:],
                              start=(kc == 0), stop=(kc == nkc - 1),
                          )
                      rc = p_es.tile([128, Tq], F32, tag="rc")
                      nc.vector.reciprocal(rc[:], cs_ps[:])
                      nc.vector.tensor_mul(y_sb[:, h, :], y_ps[:], rc[:])

              # ================ phase C: output projection ================
              if "C" not in phases:
                  continue
              with (
                  tc.tile_pool(name="cwp", bufs=2) as p_wp,
                  tc.tile_pool(name="cost", bufs=3) as p_ost,
              ):
                  for oc in range(ncc):
                      wp_oc = p_wp.tile([128, H, 128], MM_DT, tag="wp")
                      nc.sync.dma_start(wp_oc[:], Wp[oc])
                      o_ps = p_ps.tile([128, Tq], F32, tag="mm")
                      for h in range(H):
                          nc.tensor.matmul(
                              o_ps[:], wp_oc[:, h, :], y_sb[:, h, :],
                              start=(h == 0), stop=(h == H - 1),
                          )
                      ost = p_ost.tile([128, Tq], F32, tag="ost")
                      nc.scalar.copy(ost[:], o_ps[:])
                      nc.gpsimd.dma_start(outT[oc * 128:(oc + 1) * 128, :], ost[:])

    nc.compile()
    return nc


def _prep_inputs(x, W_attn, W_proj, rope_cos, rope_sin, B, T, C, Tq):
    """Host-side shard prep. Returns in_maps for the 8 cores."""
    import ml_dtypes
    mmnp = (np.float32 if MM_DT == F32R else ml_dtypes.bfloat16)
    D = 128
    H = C // D
    nkc = T // 128
    ncc = C // 128
    nvg = C // 256

    # per-head rope permutation of q/k columns: (evens | odds)
    perm = np.concatenate([np.arange(0, D, 2), np.arange(1, D, 2)])
    colperm = np.concatenate([h * D + perm for h in range(H)])

    def tile_w(w, gw):  # [C, X*gw] -> [X, 128, ncc, gw]
        return np.ascontiguousarray(
            w.reshape(ncc, 128, w.shape[1] // gw, gw).transpose(2, 1, 0, 3)
        ).astype(mmnp)

    Wq_t = tile_w(W_attn[:, 0:C][:, colperm], 128)
    Wk_t = tile_w(W_attn[:, C:2 * C][:, colperm], 128)
    Wv_t = tile_w(W_attn[:, 2 * C:3 * C], 256)
    Wp_t = tile_w(W_proj, 128)

    # rope tables in permuted layout: rows 0:64 real-pair, 64:128 imag-pair
    cosT = rope_cos.T.astype(np.float32)  # [64, T]
    sinT = rope_sin.T.astype(np.float32)
    cc = np.concatenate([cosT, cosT], axis=0)
    ss = np.concatenate([-sinT, sinT], axis=0)

    cores_per_b = N_CORES // B
    in_maps = []
    for m in range(N_CORES):
        b = m // cores_per_b
        q0 = (m % cores_per_b) * Tq
        prm = np.roll(np.arange(T), -q0)          # this core's token order
        xTf = np.ascontiguousarray(x[b].T[:, prm])  # [C, T] fp32, q tokens first
        msk = (prm[:, None] <= prm[None, 0:Tq]).astype(np.float32)
        msk = np.ascontiguousarray(msk.reshape(nkc, 128, Tq).transpose(1, 0, 2))
        in_maps.append({
            "xT": xTf.astype(mmnp),
            "Wq": Wq_t, "Wk": Wk_t, "Wv": Wv_t, "Wp": Wp_t,
            "cc": np.ascontiguousarray(cc[:, prm]).astype(mmnp),
            "ss": np.ascontiguousarray(ss[:, prm]).astype(mmnp),
            "mask": msk.astype(mmnp),
            "ones": np.ones((128, 128), dtype=mmnp),
        })
    return in_maps


_NC_CACHE = {}


def run(x, W_attn, W_proj, rope_cos, rope_sin, attention_mask=None, trace=False):
    B, T, C = x.shape
    Tq = B * T // N_CORES
    key = (B, T, C)
    if key not in _NC_CACHE:
        _NC_CACHE[key] = build_nc(B, T, C, Tq)
    nc = _NC_CACHE[key]
    in_maps = _prep_inputs(
        np.asarray(x, dtype=np.float32),
        np.asarray(W_attn, dtype=np.float32),
        np.asarray(W_proj, dtype=np.float32),
        np.asarray(rope_cos, dtype=np.float32),
        np.asarray(rope_sin, dtype=np.float32),
        B, T, C, Tq,
    )
    res = run_bass_kernel_spmd(nc, in_maps, list(range(N_CORES)), trace=trace)
    cores_per_b = N_CORES // B
    out = np.empty((B, T, C), dtype=np.float32)
    for m in range(N_CORES):
        b = m // cores_per_b
        q0 = (m % cores_per_b) * Tq
        out[b, q0:q0 + Tq, :] = res.results[m]["outT"].T
    return out, res


def kernel(x, W_attn, W_proj, rope_cos, rope_sin, attention_mask):
    out, _ = run(x, W_attn, W_proj, rope_cos, rope_sin)
    return out



# revision 3
# speedup vs baseline: 6348.1034x; 6348.1034x over previous
"""Causal self-attention (RoPE) TRN2 kernel — head-tensor-parallel, fused.

Sharding: core m = (batch m//4, heads 4*(m%4)..+4). Each core projects
q/k/v for its 4 heads over the full sequence, runs causal attention, and
applies its row-slice of W_proj, producing a PARTIAL output [C, T] f32.
Host sums 4 partials per batch (the post-c_proj all-reduce) and transposes.

Single fused instruction stream per core:
  [v proj] [h0: q,k proj+rope -> attention] [h1] [h2] [h3] [out proj]
so ACT (exp) and DVE (rope/softmax) work of head h hides under PE matmuls
of head h+1.

Causality at element granularity: for query tile qt (512 wide), key block
kc is skipped entirely above the diagonal, S/exp/PV are computed only on
the valid column range [i*128:) for diagonal-strip blocks (i = kc-4qt),
and the single triangular 128x128 self-block gets a mask multiply.

Softmax denominator: es blocks accumulate into two partial sums (even kc
on DVE, odd kc on Pool — independent chains, no cross-engine ping-pong),
combined + partition-summed by one ones-matmul per (h, qt), deferred by
one unit so PE never waits on the accumulate chain.
"""

import sys

sys.path.insert(0, "/opt/trn_rl_repo")

import numpy as np

import concourse.bacc as bacc
import concourse.mybir as mybir
import concourse.tile as tile
from concourse.bass_utils import run_bass_kernel_spmd

F32 = mybir.dt.float32
BF16 = mybir.dt.bfloat16
FP8 = mybir.dt.float8e4
MM_DT = BF16
N_CORES = 8
QT = 512


def build_nc(B, T, C, repeat=1):
    D = 128
    Hc = (C // D) * B // N_CORES   # 4 heads per core
    ncc = C // 128
    nkc = T // 128
    nqt = T // QT
    ndg = QT // 128

    nc = bacc.Bacc(None)
    xT = nc.dram_tensor("xT", [128, ncc, T], MM_DT, kind="ExternalInput")
    Wq = nc.dram_tensor("Wq", [128, Hc, ncc, 128], MM_DT, kind="ExternalInput")
    Wk = nc.dram_tensor("Wk", [128, Hc, ncc, 128], MM_DT, kind="ExternalInput")
    Wv = nc.dram_tensor("Wv", [128, ncc, Hc * 128], MM_DT, kind="ExternalInput")
    Wp = nc.dram_tensor("Wp", [128, Hc, ncc, 128], MM_DT, kind="ExternalInput")
    cc = nc.dram_tensor("cc", [128, T], BF16, kind="ExternalInput")
    ss = nc.dram_tensor("ss", [128, T], BF16, kind="ExternalInput")
    tri = nc.dram_tensor("tri", [128, 128], MM_DT, kind="ExternalInput")
    ones = nc.dram_tensor("ones", [128, 128], MM_DT, kind="ExternalInput")
    po = nc.dram_tensor("po", [C, T], F32, kind="ExternalOutput")

    scale = 1.0 / float(np.sqrt(np.float32(D)))

    with tile.TileContext(nc) as tc:
      from contextlib import ExitStack
      with ExitStack() as tp:
        p_yv = tp.enter_context(tc.tile_pool(name="yv", bufs=1))
        p_qk = tp.enter_context(tc.tile_pool(name="qk", bufs=2))
        # PSUM budget (8 banks): proj/phC 3 + S 2 + y 2 + cs 1 = 8
        p_ps = tp.enter_context(tc.tile_pool(name="ps", bufs=3, space="PSUM"))
        p_S = tp.enter_context(tc.tile_pool(name="S", bufs=2, space="PSUM"))
        p_yps = tp.enter_context(tc.tile_pool(name="yps", bufs=2, space="PSUM"))
        p_cs = tp.enter_context(tc.tile_pool(name="cs", bufs=1, space="PSUM"))
        for _rep in range(repeat):
          y_sb = p_yv.tile([128, Hc, T], MM_DT, tag="y")
          v_sb = p_yv.tile([128, nkc, Hc * 128], MM_DT, tag="v")

          with (
              tc.tile_pool(name="tabs", bufs=1) as p_tabs,
              tc.tile_pool(name="wstr", bufs=2) as p_w,
              tc.tile_pool(name="wp", bufs=1) as p_wp,
              tc.tile_pool(name="es", bufs=nkc + 1) as p_es,
              tc.tile_pool(name="dn", bufs=2) as p_dn,
          ):
            wp_sb = p_wp.tile([128, Hc, ncc, 128], MM_DT, tag="wp")
            cc_sb = p_tabs.tile([128, T], BF16, tag="cc")
            ss_sb = p_tabs.tile([128, T], BF16, tag="ss")
            tri_sb = p_tabs.tile([128, 128], MM_DT, tag="tri")
            ones_sb = p_tabs.tile([128, 128], MM_DT, tag="ones")

            state = {"pend": None}

            def flush_pend():
                h_, qsl_, yps_, esA_, esB_, bc0 = state["pend"]
                fin = p_dn.tile([128, QT], MM_DT, tag="fin")
                if bc0 > 0:
                    nc.vector.tensor_copy(fin[:, 0:bc0], esA_[:, 0:bc0])
                nc.vector.tensor_add(
                    fin[:, bc0:QT], esA_[:, bc0:QT], esB_[:, bc0:QT])
                csps = p_cs.tile([128, QT], F32, tag="cs")
                nc.tensor.matmul(csps[:], ones_sb[:], fin[:],
                                 start=True, stop=True)
                rc = p_dn.tile([128, QT], F32, tag="rc")
                nc.vector.reciprocal(rc[:], csps[:])
                nc.vector.tensor_mul(y_sb[:, h_, qsl_], yps_[:], rc[:])

            def attn_head(h, qh, kh, fill_stream, rate):
                """Causal attention for head h; drains fill_stream between
                blocks at `rate` items per block."""
                credit = [0.0]

                def drain(n):
                    if fill_stream is None:
                        return
                    credit[0] += n
                    while credit[0] >= 1.0:
                        fn = next(fill_stream, None)
                        if fn is None:
                            credit[0] = 0.0
                            return
                        fn()
                        credit[0] -= 1.0

                for qt in range(nqt):
                    nkv = ndg * (qt + 1)
                    qsl = slice(qt * QT, (qt + 1) * QT)
                    esA = p_dn.tile([128, QT], F32, tag="esA")
                    esB = p_dn.tile([128, QT], MM_DT, tag="esB")
                    esB_c0 = None  # first valid col of esB
                    es_list = []
                    for kc in range(nkv):
                        i = kc - ndg * qt
                        c0 = max(0, i) * 128  # valid cols [c0:QT)
                        sps = p_S.tile([128, QT], F32, tag="S")
                        nc.tensor.matmul(
                            sps[:, c0:QT], kh[:, kc * 128:(kc + 1) * 128],
                            qh[:, qt * QT + c0:(qt + 1) * QT],
                            start=True, stop=True,
                        )
                        es = p_es.tile([128, QT], MM_DT, tag="es")
                        nc.scalar.activation(
                            es[:, c0:QT], sps[:, c0:QT],
                            mybir.ActivationFunctionType.Exp, scale=scale,
                        )
                        if i >= 0:  # triangular self-block
                            nc.vector.tensor_mul(
                                es[:, c0:c0 + 128], es[:, c0:c0 + 128],
                                tri_sb[:],
                            )
                        if kc == 0:
                            nc.vector.tensor_copy(esA[:], es[:])
                        elif kc % 2 == 0:
                            nc.vector.tensor_add(
                                esA[:, c0:QT], esA[:, c0:QT], es[:, c0:QT])
                        elif esB_c0 is None:
                            esB_c0 = c0
                            nc.gpsimd.tensor_copy(esB[:, c0:QT], es[:, c0:QT])
                        else:
                            nc.gpsimd.tensor_add(
                                esB[:, c0:QT], esB[:, c0:QT], es[:, c0:QT])
                        es_list.append((es, c0))
                        drain(rate)
                    if state["pend"] is not None:
                        flush_pend()
                    yps = p_yps.tile([128, QT], F32, tag="y")
                    for kc, (es, c0) in enumerate(es_list):
                        nc.tensor.matmul(
                            yps[:, c0:QT],
                            v_sb[:, kc, h * 128:(h + 1) * 128],
                            es[:, c0:QT],
                            start=(kc == 0), stop=(kc == nkv - 1),
                        )
                        drain(rate)
                    state["pend"] = (h, qsl, yps, esA, esB, esB_c0 or 0)

            # ---------- phase C emission, as a drainable stream ----------
            def phc_stream(p_ost):
                gidx = 0
                for qtp in range(nqt // 2):
                    for oc in range(ncc):
                        ops = [p_ps.tile([128, QT], F32, tag="mm",
                                         name=f"opc{gidx}_{j}")
                               for j in range(2)]

                        def grp(ops=ops, oc=oc, qtp=qtp, gidx=gidx):
                            for hh in range(Hc):
                                for j in range(2):
                                    qt = qtp * 2 + j
                                    nc.tensor.matmul(
                                        ops[j][:], wp_sb[:, hh, oc, :],
                                        y_sb[:, hh, qt * QT:(qt + 1) * QT],
                                        start=(hh == 0), stop=(hh == Hc - 1),
                                    )
                            for j in range(2):
                                qt = qtp * 2 + j
                                ost = p_ost.tile([128, QT], F32, tag="ost")
                                if gidx % 2 == 0:
                                    nc.vector.tensor_copy(ost[:], ops[j][:])
                                else:
                                    nc.scalar.copy(ost[:], ops[j][:])
                                seng = (nc.sync if (gidx + j) % 2 == 0
                                        else nc.gpsimd)
                                seng.dma_start(
                                    po[oc * 128:(oc + 1) * 128,
                                       qt * QT:(qt + 1) * QT], ost[:])
                        yield grp
                        gidx += 1

            w_tiles = {}
            qk_tiles = {}

            def fetch_w(hh):
                if hh >= Hc:
                    return
                wqh = p_w.tile([128, ncc, 128], MM_DT, tag="wqh")
                nc.sync.dma_start(wqh[:], Wq[:, hh])
                wkh = p_w.tile([128, ncc, 128], MM_DT, tag="wkh")
                nc.sync.dma_start(wkh[:], Wk[:, hh])
                w_tiles[hh] = (wqh, wkh)

            with (
                tc.tile_pool(name="xw", bufs=1) as p_xw,
                tc.tile_pool(name="rope", bufs=2) as p_rope,
            ):
                # ---- DMAs: x token-chunked so v-proj starts right away ----
                wv_sb = p_xw.tile([128, ncc, Hc * 128], MM_DT, tag="wv")
                qv = ncc // 4
                for i in range(4):
                    nc.scalar.dma_start(wv_sb[:, i * qv:(i + 1) * qv, :],
                                        Wv[:, i * qv:(i + 1) * qv, :])
                x_sb = p_xw.tile([128, ncc, T], MM_DT, tag="x")
                ntk = 8
                tkw = T // ntk
                for tk in range(ntk):
                    eng = nc.sync if tk % 2 == 0 else nc.scalar
                    eng.dma_start(
                        x_sb[:, :, tk * tkw:(tk + 1) * tkw],
                        xT[:, :, tk * tkw:(tk + 1) * tkw])
                nc.sync.dma_start(cc_sb[:], cc[:])
                nc.sync.dma_start(ss_sb[:], ss[:])
                nc.sync.dma_start(tri_sb[:], tri[:])
                nc.sync.dma_start(ones_sb[:], ones[:])

                # ---- v projection (tokens on partitions), block pairs ----
                for pr in range(nkc // 2):
                    vps = [p_ps.tile([128, Hc * 128], F32, tag="mm",
                                     name=f"vps{j}") for j in range(2)]
                    for c in range(ncc):
                        for j in range(2):
                            tb = pr * 2 + j
                            nc.tensor.matmul(
                                vps[j][:], x_sb[:, c, tb * 128:(tb + 1) * 128],
                                wv_sb[:, c, :],
                                start=(c == 0), stop=(c == ncc - 1),
                            )
                    for j in range(2):
                        nc.vector.tensor_copy(v_sb[:, pr * 2 + j, :], vps[j][:])

                def rope(dst_ap, src_ps, cc_t, ss_t):
                    """dst = src*cc + swap64(src)*ss; src stays in PSUM."""
                    sw = p_rope.tile([128, QT], BF16, tag="rp_sw")
                    nc.scalar.copy(sw[0:64, :], src_ps[64:128, :])
                    nc.scalar.copy(sw[64:128, :], src_ps[0:64, :])
                    nc.gpsimd.tensor_mul(sw[:], sw[:], ss_t)
                    nc.vector.tensor_mul(dst_ap, src_ps[:], cc_t)
                    nc.vector.tensor_add(dst_ap, dst_ap, sw[:])

                def proj_stream(hh):
                    """Yield emit-callbacks for head hh's k/q projection."""
                    if hh >= Hc:
                        return
                    qh = p_qk.tile([128, T], MM_DT, tag="qh", name=f"qh{hh}")
                    kh = p_qk.tile([128, T], MM_DT, tag="kh", name=f"kh{hh}")
                    qk_tiles[hh] = (qh, kh)
                    wqh, wkh = w_tiles.pop(hh)
                    for w_sb, dst in ((wkh, kh), (wqh, qh)):
                        for pr in range(nqt // 2):
                            ps = [p_ps.tile([128, QT], F32, tag="mm",
                                            name=f"ps{hh}_{j}")
                                  for j in range(2)]
                            for c in range(ncc):
                                for j in range(2):
                                    tt = pr * 2 + j

                                    def mm(w_sb=w_sb, ps_t=ps[j], c=c, tt=tt):
                                        nc.tensor.matmul(
                                            ps_t[:], w_sb[:, c, :],
                                            x_sb[:, c,
                                                 tt * QT:(tt + 1) * QT],
                                            start=(c == 0),
                                            stop=(c == ncc - 1),
                                        )
                                    yield mm
                            for j in range(2):
                                tt = pr * 2 + j
                                sl = slice(tt * QT, (tt + 1) * QT)

                                def rp(dst=dst, ps_t=ps[j], sl=sl):
                                    rope(dst[:, sl], ps_t,
                                         cc_sb[:, sl], ss_sb[:, sl])
                                yield rp

                fetch_w(0)
                fetch_w(1)
                for fn in proj_stream(0):
                    fn()

                for h in range(Hc - 1):
                    fetch_w(h + 2)
                    pstream = proj_stream(h + 1)
                    qh, kh = qk_tiles[h]
                    attn_head(h, qh, kh, pstream, 1.6)
                    for fn in pstream:  # leftover proj of h+1
                        fn()

            # x/wv/rope freed; last head's attention drains phase C qtp0
            with tc.tile_pool(name="ost", bufs=6) as p_ost:
                for chk in range(4):
                    nc.sync.dma_start(
                        wp_sb[:, :, chk * 4:(chk + 1) * 4, :],
                        Wp[:, :, chk * 4:(chk + 1) * 4, :])
                phcs = phc_stream(p_ost)
                h = Hc - 1
                qh, kh = qk_tiles[h]
                # Only the ncc qtp0 groups are dep-safe during the last
                # head's attention (they read y[*, qt0/qt1] only), and only
                # once the (h3, qt1) flush has been EMITTED — i.e. from
                # qt2's PV phase onward. Anything more would enqueue a PE
                # matmul that waits on a flush emitted later in the same
                # in-order queue (deadlock).
                credit = [0.0]
                emitted = [0]

                def fill(n, qt, pv):
                    if not (qt > 2 or (qt == 2 and pv)):
                        return
                    if emitted[0] >= ncc:
                        return
                    credit[0] += n
                    while credit[0] >= 1.0 and emitted[0] < ncc:
                        fn = next(phcs, None)
                        if fn is None:
                            credit[0] = 0.0
                            return
                        fn()
                        emitted[0] += 1
                        credit[0] -= 1.0

                # inline attention for the last head with phC filler
                for qt in range(nqt):
                    nkv = ndg * (qt + 1)
                    qsl = slice(qt * QT, (qt + 1) * QT)
                    esA = p_dn.tile([128, QT], F32, tag="esA")
                    esB = p_dn.tile([128, QT], MM_DT, tag="esB")
                    esB_c0 = None
                    es_list = []
                    for kc in range(nkv):
                        i = kc - ndg * qt
                        c0 = max(0, i) * 128
                        sps = p_S.tile([128, QT], F32, tag="S")
                        nc.tensor.matmul(
                            sps[:, c0:QT], kh[:, kc * 128:(kc + 1) * 128],
                            qh[:, qt * QT + c0:(qt + 1) * QT],
                            start=True, stop=True,
                        )
                        es = p_es.tile([128, QT], MM_DT, tag="es")
                        nc.scalar.activation(
                            es[:, c0:QT], sps[:, c0:QT],
                            mybir.ActivationFunctionType.Exp, scale=scale,
                        )
                        if i >= 0:
                            nc.vector.tensor_mul(
                                es[:, c0:c0 + 128], es[:, c0:c0 + 128],
                                tri_sb[:],
                            )
                        if kc == 0:
                            nc.vector.tensor_copy(esA[:], es[:])
                        elif kc % 2 == 0:
                            nc.vector.tensor_add(
                                esA[:, c0:QT], esA[:, c0:QT], es[:, c0:QT])
                        elif esB_c0 is None:
                            esB_c0 = c0
                            nc.gpsimd.tensor_copy(esB[:, c0:QT], es[:, c0:QT])
                        else:
                            nc.gpsimd.tensor_add(
                                esB[:, c0:QT], esB[:, c0:QT], es[:, c0:QT])
                        es_list.append((es, c0))
                        fill(0.5, qt, False)
                    if state["pend"] is not None:
                        flush_pend()
                    yps = p_yps.tile([128, QT], F32, tag="y")
                    for kc, (es, c0) in enumerate(es_list):
                        nc.tensor.matmul(
                            yps[:, c0:QT],
                            v_sb[:, kc, h * 128:(h + 1) * 128],
                            es[:, c0:QT],
                            start=(kc == 0), stop=(kc == nkv - 1),
                        )
                        fill(0.5, qt, True)
                    state["pend"] = (h, qsl, yps, esA, esB, esB_c0 or 0)
                flush_pend()
                state["pend"] = None

                # ---- rest of the partial output projection ----
                for fn in phcs:
                    fn()

    nc.compile()
    return nc


def _prep_inputs(x, W_attn, W_proj, rope_cos, rope_sin, B, T, C):
    import ml_dtypes
    mmnp = ml_dtypes.bfloat16
    D = 128
    H = C // D
    Hc = H * B // N_CORES
    ncc = C // 128

    perm = np.concatenate([np.arange(0, D, 2), np.arange(1, D, 2)])
    cosT = rope_cos.T.astype(np.float32)
    sinT = rope_sin.T.astype(np.float32)
    cc = np.concatenate([cosT, cosT], axis=0).astype(mmnp)
    ss = np.concatenate([-sinT, sinT], axis=0).astype(mmnp)

    tri = (np.arange(128)[:, None] <= np.arange(128)[None, :]).astype(mmnp)
    trib = np.where(np.arange(128)[:, None] <= np.arange(128)[None, :],
                    0.0, -1e30).astype(np.float32)
    ones = np.ones((128, 128), dtype=mmnp)

    xTs = [
        np.ascontiguousarray(
            x[b].T.reshape(ncc, 128, T).transpose(1, 0, 2)).astype(mmnp)
        for b in range(B)
    ]

    def stat_tiles(w):  # [C, Hc*128] -> [128, Hc, ncc, 128]
        return np.ascontiguousarray(
            w.reshape(ncc, 128, Hc, 128).transpose(1, 2, 0, 3)).astype(mmnp)

    groups = []
    for g in range(N_CORES // B):
        hsl = np.arange(g * Hc * D, (g + 1) * Hc * D)
        cperm = np.concatenate([g * Hc * D + h * D + perm for h in range(Hc)])
        Wq_t = stat_tiles(W_attn[:, 0:C][:, cperm])
        Wk_t = stat_tiles(W_attn[:, C:2 * C][:, cperm])
        Wv_t = np.ascontiguousarray(
            W_attn[:, 2 * C:3 * C][:, hsl]
            .reshape(ncc, 128, Hc * 128).transpose(1, 0, 2)).astype(mmnp)
        Wp_t = np.ascontiguousarray(
            W_proj[hsl, :].reshape(Hc, 128, ncc, 128)
            .transpose(1, 0, 2, 3)).astype(mmnp)
        groups.append((Wq_t, Wk_t, Wv_t, Wp_t))

    in_maps = []
    for m in range(N_CORES):
        b = m // (N_CORES // B)
        Wq_t, Wk_t, Wv_t, Wp_t = groups[m % (N_CORES // B)]
        in_maps.append({
            "xT": xTs[b], "Wq": Wq_t, "Wk": Wk_t, "Wv": Wv_t, "Wp": Wp_t,
            "cc": cc, "ss": ss, "tri": tri, "trib": trib, "ones": ones,
        })
    return in_maps


_NC_CACHE = {}


def run(x, W_attn, W_proj, rope_cos, rope_sin, attention_mask=None, trace=False):
    B, T, C = x.shape
    key = (B, T, C)
    if key not in _NC_CACHE:
        _NC_CACHE[key] = build_nc(B, T, C)
    nc = _NC_CACHE[key]
    in_maps = _prep_inputs(
        np.asarray(x, dtype=np.float32),
        np.asarray(W_attn, dtype=np.float32),
        np.asarray(W_proj, dtype=np.float32),
        np.asarray(rope_cos, dtype=np.float32),
        np.asarray(rope_sin, dtype=np.float32),
        B, T, C,
    )
    res = run_bass_kernel_spmd(nc, in_maps, list(range(N_CORES)), trace=trace)
    gpb = N_CORES // B
    out = np.empty((B, T, C), dtype=np.float32)
    for b in range(B):
        acc = res.results[b * gpb]["po"].astype(np.float64)
        for j in range(1, gpb):
            acc += res.results[b * gpb + j]["po"]
        out[b] = acc.T
    return out, res


def kernel(x, W_attn, W_proj, rope_cos, rope_sin, attention_mask):
    out, _ = run(x, W_attn, W_proj, rope_cos, rope_sin)
    return out
